# revision 38
# baseline (speedup 1.0000x reference)
"""LoLa message-passing kernel for 8 Trainium2 NeuronCores.

Math (algebraically identical to the reference):
  ch0 masses      = f3^2 - f0^2 - f1^2 - f2^2
  ch1 ptsq        = f1^2 + f2^2
  ch2 w_ener@f0, ch4 w_pid@f3, ch5 w_extra0@f4, ch6 w_extra1@f5
  ch3 weighted_d  = masses * rowsum(w_dist) + w_dist @ masses
                    + 2*(f0*(w_dist@f0) + f1*(w_dist@f1)
                         + f2*(w_dist@f2) - f3*(w_dist@f3))

Sharding: model-parallel over particles N (64 output rows per core); combvec
replicated (full contraction operand), weights sliced 1/8 per core.

Device-side design notes:
 - Single-term bf16 everywhere (fp32 PSUM accumulate): rel err ~3.6e-3,
   well under the 2e-2 gate, at half the HBM bytes and a third of the
   matmul count of an fp32-faithful hi/lo split.
 - The input stream is byte-rate-bound (~160 GB/s aggregate with all 8
   cores pulling, regardless of queue count or descriptor size). The two
   HWDGE queues are byte-balanced (589/588 KB) and ordered so weights
   land first (every matmul needs them) and ft chunk 2 lands last.
 - Stationary pairs pack two 64-row weight slices side by side; w_dist is
   stored once (C2 reuses pair 0's stationary):
     MM-A: [w_dist | w_ener]  @ [f0|f1|f2|f3]        (512 cols -> psA)
     MM-B: [w_pid  | w_extra0]@ [f3|f4]              (256 cols -> psB)
     MM-C1: w_extra1 (64-col stationary) @ f5        (128 cols -> psC1 lo)
     MM-C2: [w_dist | w_ener] @ [masses|1]           (129 cols -> psC2)
 - masses per 128-row chunk: square + 3 chained subtracts in bf16, written
   straight into the mt tile. Chunks 0+1 as one strided 2-chunk batch on
   vector; chunks 2/3 squares on the scalar ACT engine, combines on
   vector/gpsimd (tensor_reduce measured slower than tt chains).
 - fr (this core's 64 rows of f0..f3) ships bf16 with f3 negated on the
   host, so this core's masses/ptsq squares and the quad combine are
   sign-uniform adds.
 - tc.tile_wait_until stamps force the Tile scheduler's per-engine order
   to match real DMA arrival times (its cost model assumes instant DMAs
   and otherwise schedules the quad chain ahead of the masses combines
   that gate psC2).
"""

import sys

if "/opt/trn_rl_repo" not in sys.path:
    sys.path.insert(0, "/opt/trn_rl_repo")

import numpy as np
import ml_dtypes

import concourse.bass as bass
import concourse.mybir as mybir
import concourse.tile as tile
from concourse import bacc
from concourse.bass_utils import run_bass_kernel_spmd

B, N, F = 128, 512, 6
NCORES = 8
NS = N // NCORES  # 64 output rows per core
KC = N // 128  # 4 contraction chunks of 128
FW = 768  # ft cols per chunk: 6 features x 128 batch
MW = 132  # mt cols per chunk: 128 masses | 1 one | 3 pad
PW = 320  # wt cols per chunk: [w_dist|w_ener] 128, [w_pid|w_x0] 128, w_x1 64
DT = mybir.dt.float32
BF = mybir.dt.bfloat16
ALU = mybir.AluOpType
ACTF = mybir.ActivationFunctionType


def _emit(tc, nc, ft_d, wt_d, fr_d, out_d):
    with (
        tc.tile_pool(name="sbuf", bufs=1) as sb,
        tc.tile_pool(name="scratch", bufs=4) as scr,
        tc.tile_pool(name="psum", bufs=1, space="PSUM") as ps,
    ):
        # --- persistent SBUF tiles ---
        ft = sb.tile([128, KC * FW], BF)  # features [c*768 + k*128 + b]
        mt = sb.tile([128, KC * MW], BF)  # masses [c*132 + b], ones at c*132+128
        wt = sb.tile([128, KC * PW], BF)  # weight pairs [c*320 + ...]
        fr = sb.tile([64, 4 * B], BF)  # this core's n-rows of f0..f3 (f3 negated)
        frsq = sb.tile([64, 4 * B], DT)
        quad = sb.tile([64, 4 * B], DT)
        qs = sb.tile([64, 2 * B], DT)
        tmp3 = sb.tile([64, B], DT)
        tmp3f = sb.tile([64, B], DT)
        olo = sb.tile([64, 5 * B], DT)  # out staging ch 0,1,3,4,6 (partitions 0:64)
        ohi = sb.tile([128, 2 * B], DT)  # out staging ch 2,5 (partitions 64:128)

        # --- PSUM tiles ---
        psA = ps.tile([128, 512], DT)  # [dist|ener] @ [f0|f1|f2|f3]
        psB = ps.tile([128, 256], DT)  # [pid|x0]   @ [f3|f4]
        psC1 = ps.tile([64, B], DT)  # x1 @ f5 (64-col stationary -> lo partitions)
        psC2 = ps.tile([128, 132], DT)  # [dist|ener] @ [m|1] (dist/rowsum rows :64)

        # --- DMAs in: byte-balanced across the two HWDGE queues (each
        # sustains ~85-110 GB/s; together ~160). sync: ft chunks 0+1 then 3;
        # scalar: fr (feeds the early gpsimd chain), weights, ft chunk 2. ---
        nc.sync.dma_start(ft[:, 0: 2 * FW], ft_d[:, 0: 2 * FW])
        nc.scalar.dma_start(wt[:], wt_d[:])
        nc.scalar.dma_start(fr[:], fr_d[:])
        nc.sync.dma_start(ft[:, 3 * FW:], ft_d[:, 3 * FW:])
        nc.scalar.dma_start(ft[:, 2 * FW: 3 * FW], ft_d[:, 2 * FW: 3 * FW])

        # ones column at [c*MW + 128] (vector's first op; no DMA dep)
        mt4 = mt[:].rearrange("p (c x) -> p c x", c=KC, x=MW)
        nc.vector.memset(mt4[:, :, 128:129], 1.0)

        # --- PE warm-up: dep-free dummy matmuls give HAM sustained activity
        # from kernel start until the weights land (~12us), so the clock
        # ramps (1.2 -> 2.4 GHz) before the real matmul crunch. Also
        # tightens run-to-run clock-state variance. ---
        warm = sb.tile([128, 2 * B], BF)
        psW = ps.tile([128, 512], DT)
        nc.vector.memset(warm[:], 0.5)
        wmov = warm[:, None, :].to_broadcast([128, 4, 2 * B])
        for i in range(7):
            nc.tensor.matmul(
                psW[:], warm[:, 0:B], wmov[:, :, 0:B], start=i == 0, stop=i == 6
            )

        # --- masses: sq = f*f (bf16), m = sq3-sq2-sq1-sq0 chained, last
        # subtract lands in mt. Chunks 0+1 as one strided 2-chunk batch on
        # vector (halves per-op overhead); chunk 2 on vector, chunk 3 on
        # gpsimd (free after the early fr chain). ---
        sq01 = scr.tile([128, 1024], BF, name="sq01")
        t01 = scr.tile([128, 2 * B], BF, name="t01")
        ftv = ft[:, 0: 2 * FW].rearrange("p (cc x) -> p cc x", cc=2, x=FW)
        sqv = sq01[:].rearrange("p (cc x) -> p cc x", cc=2, x=512)
        tv = t01[:].rearrange("p (cc x) -> p cc x", cc=2, x=B)
        mtv = mt[:, 0: 2 * MW].rearrange("p (cc x) -> p cc x", cc=2, x=MW)
        nc.vector.tensor_tensor(
            out=sq01[:], in0=ftv[:, :, 0:512], in1=ftv[:, :, 0:512], op=ALU.mult
        )
        nc.vector.tensor_tensor(
            out=t01[:], in0=sqv[:, :, 3 * B: 4 * B], in1=sqv[:, :, 2 * B: 3 * B],
            op=ALU.subtract,
        )
        nc.vector.tensor_tensor(
            out=t01[:], in0=t01[:], in1=sqv[:, :, B: 2 * B], op=ALU.subtract
        )
        nc.vector.tensor_tensor(
            out=mtv[:, :, 0:B], in0=tv, in1=sqv[:, :, 0:B], op=ALU.subtract
        )
        # chunks 2+3: squares on scalar ACT (otherwise idle mid-kernel);
        # combines on vector (c2) / gpsimd (c3). high_priority hints the
        # scheduler to place the combines ahead of the quad chain, which it
        # otherwise reorders in front of them (its cost model does not see
        # real DMA arrival times).
        sq2 = scr.tile([128, 4 * B], BF, name="sq2")
        t2 = scr.tile([128, B], BF, name="t2")
        sq3 = scr.tile([128, 4 * B], BF, name="sq3")
        t3 = scr.tile([128, B], BF, name="t3")
        # scalar queue order matches real arrival: ft3 (~11.5us) -> fr
        # (~12us) -> ft2 (last, ~14.4us).
        nc.scalar.activation(sq3[:], ft[:, 3 * FW: 3 * FW + 512], ACTF.Square)
        nc.scalar.activation(frsq[:], fr[:], ACTF.Square)
        nc.scalar.activation(sq2[:], ft[:, 2 * FW: 2 * FW + 512], ACTF.Square)
        # c3 combines on gpsimd (ft3 arrives early on the sync queue), then
        # this core's ch0/ch1 combines; the LAST-arriving chunk (ft2) gets
        # its combines on vector ahead of the quad chain.
        with tc.tile_wait_until(1):
            nc.gpsimd.tensor_tensor(
                out=t3[:], in0=sq3[:, 3 * B: 4 * B], in1=sq3[:, 2 * B: 3 * B],
                op=ALU.subtract,
            )
            nc.gpsimd.tensor_tensor(
                out=t3[:], in0=t3[:], in1=sq3[:, B: 2 * B], op=ALU.subtract
            )
            nc.gpsimd.tensor_tensor(
                out=mt[:, 3 * MW: 3 * MW + B], in0=t3[:], in1=sq3[:, 0:B],
                op=ALU.subtract,
            )
        with tc.tile_wait_until(2):
            nc.gpsimd.tensor_tensor(
                out=olo[:, B: 2 * B], in0=frsq[:, B: 2 * B],
                in1=frsq[:, 2 * B: 3 * B], op=ALU.add,
            )
            nc.gpsimd.tensor_tensor(
                out=tmp3[:], in0=frsq[:, 3 * B: 4 * B], in1=frsq[:, 0:B],
                op=ALU.subtract,
            )
            nc.gpsimd.tensor_tensor(
                out=olo[:, 0:B], in0=tmp3[:], in1=olo[:, B: 2 * B], op=ALU.subtract
            )
            nc.vector.tensor_tensor(
                out=t2[:], in0=sq2[:, 3 * B: 4 * B], in1=sq2[:, 2 * B: 3 * B],
                op=ALU.subtract,
            )
            nc.vector.tensor_tensor(
                out=t2[:], in0=t2[:], in1=sq2[:, B: 2 * B], op=ALU.subtract
            )
            nc.vector.tensor_tensor(
                out=mt[:, 2 * MW: 2 * MW + B], in0=t2[:], in1=sq2[:, 0:B],
                op=ALU.subtract,
            )

        # --- matmuls: A/B/C1 for all chunks first (DMA-gated only), then the
        # masses-dependent C2s (reusing pair 0's stationary). ---
        def mmABC(c):
            fb = c * FW
            wb = c * PW
            nc.tensor.matmul(
                psA[:], wt[:, wb: wb + 128], ft[:, fb: fb + 512],
                start=c == 0, stop=c == 2,
            )
            nc.tensor.matmul(
                psB[:], wt[:, wb + 128: wb + 256], ft[:, fb + 384: fb + 640],
                start=c == 0, stop=c == 2,
            )
            nc.tensor.matmul(
                psC1[:], wt[:, wb + 256: wb + 320], ft[:, fb + 640: fb + 768],
                start=c == 0, stop=c == 2,
            )

        def mmC2(c, start=False, stop=False):
            nc.tensor.matmul(
                psC2[:, 0:129], wt[:, c * PW: c * PW + 128],
                mt[:, c * MW: c * MW + 129],
                start=start, stop=stop,
            )

        # PE order follows real DMA arrival (ft chunk 3 lands before ft
        # chunk 2, which is last); C2 c0/c1 squeeze in between. Stop flags
        # sit on chunk 2's matmuls (last executed of each group).
        mmABC(0)
        mmABC(1)
        mmABC(3)
        mmC2(0, start=True)
        mmC2(1)
        mmABC(2)
        with tc.tile_wait_until(1):
            mmC2(3)
        with tc.tile_wait_until(2):
            mmC2(2, stop=True)

        # --- quad chain on vector: fr * psA rows is all-additive thanks to
        # the host-side f3 negation; tt adds + two fused stt ops finish ch3
        # (tensor_reduce measured slower than chained tt). ---
        with tc.tile_wait_until(3):
            nc.vector.tensor_tensor(
                out=quad[:], in0=fr[:], in1=psA[0:64, :], op=ALU.mult
            )
            nc.vector.tensor_tensor(
                out=qs[:, 0: 2 * B], in0=quad[:, 0: 2 * B],
                in1=quad[:, 2 * B: 4 * B], op=ALU.add,
            )
            nc.vector.tensor_tensor(
                out=qs[:, 0:B], in0=qs[:, 0:B], in1=qs[:, B: 2 * B], op=ALU.add
            )
        with tc.tile_wait_until(4):
            nc.vector.scalar_tensor_tensor(
                out=tmp3f[:],
                in0=olo[:, 0:B],
                scalar=psC2[0:64, 128:129],
                in1=psC2[0:64, 0:B],
                op0=ALU.mult,
                op1=ALU.add,
            )
            nc.vector.scalar_tensor_tensor(
                out=olo[:, 2 * B: 3 * B],
                in0=qs[:, 0:B],
                scalar=2.0,
                in1=tmp3f[:],
                op0=ALU.mult,
                op1=ALU.add,
            )
        # ch4 = w_pid@f3 and ch6 = w_x1@f5 (low partitions); ch2/ch5 (high)
        nc.scalar.copy(olo[:, 3 * B: 4 * B], psB[0:64, 0:B])
        nc.scalar.copy(olo[:, 4 * B: 5 * B], psC1[:, 0:B])
        nc.scalar.copy(ohi[64:128, 0:B], psA[64:128, 0:B])  # ch2 ener
        nc.scalar.copy(ohi[64:128, B: 2 * B], psB[64:128, B: 2 * B])  # ch5 x0

        # --- DMAs out: one per staging tile, on separate queues ---
        nc.scalar.dma_start(out_d[:, 5 * B: 7 * B], ohi[64:128, :])
        nc.sync.dma_start(out_d[:, 0: 5 * B], olo[:])


_NC_CACHE = {}


def _get_nc():
    if "nc" not in _NC_CACHE:
        nc = bacc.Bacc(
            "TRN2", target_bir_lowering=False, debug=False, num_devices=NCORES
        )
        ft_d = nc.dram_tensor("ft", [128, KC * FW], BF, kind="ExternalInput")
        wt_d = nc.dram_tensor("wt", [128, KC * PW], BF, kind="ExternalInput")
        fr_d = nc.dram_tensor("fr", [64, 4 * B], BF, kind="ExternalInput")
        out_d = nc.dram_tensor("out", [64, 7 * B], DT, kind="ExternalOutput")
        with tile.TileContext(nc) as tc:
            _emit(tc, nc, ft_d.ap(), wt_d.ap(), fr_d.ap(), out_d.ap())
        nc.compile()
        _NC_CACHE["nc"] = nc
    return _NC_CACHE["nc"]


def make_in_maps(combvec, w_dist, w_ener, w_pid, w_extra0, w_extra1):
    ft_t = np.ascontiguousarray(
        np.transpose(np.asarray(combvec, np.float32), (2, 1, 0))
    )  # (6, 512, 128) [k, m, b]
    # ft layout: [p, c*768 + k*128 + b] = ft_t[k, c*128+p, b]
    ftfull = np.ascontiguousarray(
        ft_t.reshape(F, KC, 128, B).transpose(2, 1, 0, 3)
    ).reshape(128, KC * FW)
    ft_np = ftfull.astype(ml_dtypes.bfloat16)

    weights = {
        "w_dist": np.asarray(w_dist, np.float32),
        "w_pid": np.asarray(w_pid, np.float32),
        "w_ener": np.asarray(w_ener, np.float32),
        "w_extra0": np.asarray(w_extra0, np.float32),
        "w_extra1": np.asarray(w_extra1, np.float32),
    }
    in_maps = []
    for core in range(NCORES):
        sl = slice(NS * core, NS * (core + 1))
        # per weight: slice (64, 512) -> transposed chunks (c, p, n) = (4,128,64)
        wch = {
            k: w[sl].T.reshape(KC, 128, NS) for k, w in weights.items()
        }
        # wt layout per chunk: [w_dist|w_ener] (128), [w_pid|w_x0] (128), w_x1 (64)
        wt_ = np.concatenate(
            [
                np.concatenate(
                    [wch["w_dist"], wch["w_ener"], wch["w_pid"],
                     wch["w_extra0"], wch["w_extra1"]], axis=2
                )  # (c, p, 320)
            ],
            axis=2,
        ).transpose(1, 0, 2).reshape(128, KC * PW)
        wt_np = np.ascontiguousarray(wt_).astype(ml_dtypes.bfloat16)
        # fr layout: [p, k*128 + b] = ft_t[k, 64*core+p, b], bf16, f3 negated
        frc = np.ascontiguousarray(ft_t[:4, sl, :].transpose(1, 0, 2)).copy()
        frc[:, 3, :] *= -1.0
        frc_np = frc.reshape(NS, 4 * B).astype(ml_dtypes.bfloat16)
        in_maps.append({"ft": ft_np, "wt": wt_np, "fr": frc_np})
    return in_maps


# out channel order in the DRAM out tensor columns
OUT_ORDER = [0, 1, 3, 4, 6, 2, 5]


def assemble(results):
    full = np.empty((B, N, 7), np.float32)
    for core, r in enumerate(results):
        o = r["out"].reshape(NS, 7, B)  # (n, slot, b)
        for slot, ch in enumerate(OUT_ORDER):
            full[:, NS * core: NS * (core + 1), ch] = o[:, slot, :].T
    return full


def kernel(combvec, w_dist, w_ener, w_pid, w_extra0, w_extra1, _bench=None):
    in_maps = make_in_maps(combvec, w_dist, w_ener, w_pid, w_extra0, w_extra1)
    nc = _get_nc()
    kw = dict(_bench) if _bench else {}
    res = run_bass_kernel_spmd(nc, in_maps, core_ids=list(range(NCORES)), **kw)
    out = assemble(res.results)
    if _bench is not None:
        kernel.last_results = res
    return out


# revision 41
# speedup vs baseline: 1.0064x; 1.0064x over previous
"""LoLa message-passing kernel for 8 Trainium2 NeuronCores.

Math (algebraically identical to the reference):
  ch0 masses      = f3^2 - f0^2 - f1^2 - f2^2
  ch1 ptsq        = f1^2 + f2^2
  ch2 w_ener@f0, ch4 w_pid@f3, ch5 w_extra0@f4, ch6 w_extra1@f5
  ch3 weighted_d  = masses * rowsum(w_dist) + w_dist @ masses
                    + 2*(f0*(w_dist@f0) + f1*(w_dist@f1)
                         + f2*(w_dist@f2) - f3*(w_dist@f3))

Sharding: model-parallel over particles N (64 output rows per core); combvec
replicated (full contraction operand), weights sliced 1/8 per core.

Device-side design notes:
 - Single-term bf16 everywhere (fp32 PSUM accumulate): rel err ~3.6e-3,
   well under the 2e-2 gate, at half the HBM bytes and a third of the
   matmul count of an fp32-faithful hi/lo split.
 - The input stream is byte-rate-bound (~160 GB/s aggregate with all 8
   cores pulling, regardless of queue count or descriptor size). The two
   HWDGE queues are byte-balanced (589/588 KB) and ordered so weights
   land first (every matmul needs them) and ft chunk 2 lands last.
 - Stationary pairs pack two 64-row weight slices side by side; w_dist is
   stored once (C2 reuses pair 0's stationary):
     MM-A: [w_dist | w_ener]  @ [f0|f1|f2|f3]        (512 cols -> psA)
     MM-B: [w_pid  | w_extra0]@ [f3|f4]              (256 cols -> psB)
     MM-C1: w_extra1 (64-col stationary) @ f5        (128 cols -> psC1 lo)
     MM-C2: [w_dist | w_ener] @ [masses|1]           (129 cols -> psC2)
 - masses per 128-row chunk: square + 3 chained subtracts in bf16, written
   straight into the mt tile. Chunks 0+1 as one strided 2-chunk batch on
   vector; chunks 2/3 squares on the scalar ACT engine, combines on
   vector/gpsimd (tensor_reduce measured slower than tt chains).
 - fr (this core's 64 rows of f0..f3) ships bf16 with f3 negated on the
   host, so this core's masses/ptsq squares and the quad combine are
   sign-uniform adds.
 - tc.tile_wait_until stamps force the Tile scheduler's per-engine order
   to match real DMA arrival times (its cost model assumes instant DMAs
   and otherwise schedules the quad chain ahead of the masses combines
   that gate psC2).
"""

import sys

if "/opt/trn_rl_repo" not in sys.path:
    sys.path.insert(0, "/opt/trn_rl_repo")

import numpy as np
import ml_dtypes

import concourse.bass as bass
import concourse.mybir as mybir
import concourse.tile as tile
from concourse import bacc
from concourse.bass_utils import run_bass_kernel_spmd

B, N, F = 128, 512, 6
NCORES = 8
NS = N // NCORES  # 64 output rows per core
KC = N // 128  # 4 contraction chunks of 128
FW = 768  # ft cols per chunk: 6 features x 128 batch
MW = 132  # mt cols per chunk: 128 masses | 1 one | 3 pad
PW = 320  # wt cols per chunk: [w_dist|w_ener] 128, [w_pid|w_x0] 128, w_x1 64
DT = mybir.dt.float32
BF = mybir.dt.bfloat16
ALU = mybir.AluOpType
ACTF = mybir.ActivationFunctionType


def _emit(tc, nc, ft_d, wt_d, fr_d, out_d):
    with (
        tc.tile_pool(name="sbuf", bufs=1) as sb,
        tc.tile_pool(name="scratch", bufs=4) as scr,
        tc.tile_pool(name="psum", bufs=1, space="PSUM") as ps,
    ):
        # --- persistent SBUF tiles ---
        ft = sb.tile([128, KC * FW], BF)  # features [c*768 + k*128 + b]
        mt = sb.tile([128, KC * MW], BF)  # masses [c*132 + b], ones at c*132+128
        wt = sb.tile([128, KC * PW], BF)  # weight pairs [c*320 + ...]
        fr = sb.tile([64, 4 * B], BF)  # this core's n-rows of f0..f3 (f3 negated)
        frsq = sb.tile([64, 4 * B], DT)
        quad = sb.tile([64, 4 * B], BF)  # bf16: 2x DVE rate; feeds 2*qsum only
        qs = sb.tile([64, 2 * B], BF)
        tmp3 = sb.tile([64, B], DT)
        tmp3f = sb.tile([64, B], DT)
        olo = sb.tile([64, 5 * B], DT)  # out staging ch 0,1,3,4,6 (partitions 0:64)
        ohi = sb.tile([128, 2 * B], DT)  # out staging ch 2,5 (partitions 64:128)

        # --- PSUM tiles ---
        psA = ps.tile([128, 512], DT)  # [dist|ener] @ [f0|f1|f2|f3]
        psB = ps.tile([128, 256], DT)  # [pid|x0]   @ [f3|f4]
        psC1 = ps.tile([64, B], DT)  # x1 @ f5 (64-col stationary -> lo partitions)
        psC2 = ps.tile([128, 132], DT)  # [dist|ener] @ [m|1] (dist/rowsum rows :64)

        # --- DMAs in: byte-balanced across the two HWDGE queues (each
        # sustains ~85-110 GB/s; together ~160). sync: ft chunks 0+1 then 3;
        # scalar: fr (feeds the early gpsimd chain), weights, ft chunk 2. ---
        nc.sync.dma_start(ft[:, 0: 2 * FW], ft_d[:, 0: 2 * FW])
        nc.scalar.dma_start(wt[:], wt_d[:])
        nc.scalar.dma_start(fr[:], fr_d[:])
        nc.sync.dma_start(ft[:, 3 * FW:], ft_d[:, 3 * FW:])
        nc.scalar.dma_start(ft[:, 2 * FW: 3 * FW], ft_d[:, 2 * FW: 3 * FW])

        # ones column at [c*MW + 128] (vector's first op; no DMA dep)
        mt4 = mt[:].rearrange("p (c x) -> p c x", c=KC, x=MW)
        nc.vector.memset(mt4[:, :, 128:129], 1.0)

        # --- PE warm-up: dep-free dummy matmuls give HAM sustained activity
        # from kernel start until the weights land (~12us), so the clock
        # ramps (1.2 -> 2.4 GHz) before the real matmul crunch. Also
        # tightens run-to-run clock-state variance. ---
        warm = sb.tile([128, 2 * B], BF)
        psW = ps.tile([128, 512], DT)
        nc.vector.memset(warm[:], 0.5)
        wmov = warm[:, None, :].to_broadcast([128, 4, 2 * B])
        for i in range(7):
            nc.tensor.matmul(
                psW[:], warm[:, 0:B], wmov[:, :, 0:B], start=i == 0, stop=i == 6
            )

        # --- masses: sq = f*f (bf16), m = sq3-sq2-sq1-sq0 chained, last
        # subtract lands in mt. Chunks 0+1 as one strided 2-chunk batch on
        # vector (halves per-op overhead); chunk 2 on vector, chunk 3 on
        # gpsimd (free after the early fr chain). ---
        sq01 = scr.tile([128, 1024], BF, name="sq01")
        t01 = scr.tile([128, 2 * B], BF, name="t01")
        ftv = ft[:, 0: 2 * FW].rearrange("p (cc x) -> p cc x", cc=2, x=FW)
        sqv = sq01[:].rearrange("p (cc x) -> p cc x", cc=2, x=512)
        tv = t01[:].rearrange("p (cc x) -> p cc x", cc=2, x=B)
        mtv = mt[:, 0: 2 * MW].rearrange("p (cc x) -> p cc x", cc=2, x=MW)
        nc.vector.tensor_tensor(
            out=sq01[:], in0=ftv[:, :, 0:512], in1=ftv[:, :, 0:512], op=ALU.mult
        )
        nc.vector.tensor_tensor(
            out=t01[:], in0=sqv[:, :, 3 * B: 4 * B], in1=sqv[:, :, 2 * B: 3 * B],
            op=ALU.subtract,
        )
        nc.vector.tensor_tensor(
            out=t01[:], in0=t01[:], in1=sqv[:, :, B: 2 * B], op=ALU.subtract
        )
        nc.vector.tensor_tensor(
            out=mtv[:, :, 0:B], in0=tv, in1=sqv[:, :, 0:B], op=ALU.subtract
        )
        # chunks 2+3: squares on scalar ACT (otherwise idle mid-kernel);
        # combines on vector (c2) / gpsimd (c3). high_priority hints the
        # scheduler to place the combines ahead of the quad chain, which it
        # otherwise reorders in front of them (its cost model does not see
        # real DMA arrival times).
        sq2 = scr.tile([128, 4 * B], BF, name="sq2")
        t2 = scr.tile([128, B], BF, name="t2")
        sq3 = scr.tile([128, 4 * B], BF, name="sq3")
        t3 = scr.tile([128, B], BF, name="t3")
        # scalar queue order matches real arrival: ft3 (~11.5us) -> fr
        # (~12us) -> ft2 (last, ~14.4us).
        nc.scalar.activation(sq3[:], ft[:, 3 * FW: 3 * FW + 512], ACTF.Square)
        nc.scalar.activation(frsq[:], fr[:], ACTF.Square)
        nc.scalar.activation(sq2[:], ft[:, 2 * FW: 2 * FW + 512], ACTF.Square)
        # c3 combines on gpsimd (ft3 arrives early on the sync queue), then
        # this core's ch0/ch1 combines; the LAST-arriving chunk (ft2) gets
        # its combines on vector ahead of the quad chain.
        with tc.tile_wait_until(1):
            nc.gpsimd.tensor_tensor(
                out=t3[:], in0=sq3[:, 3 * B: 4 * B], in1=sq3[:, 2 * B: 3 * B],
                op=ALU.subtract,
            )
            nc.gpsimd.tensor_tensor(
                out=t3[:], in0=t3[:], in1=sq3[:, B: 2 * B], op=ALU.subtract
            )
            nc.gpsimd.tensor_tensor(
                out=mt[:, 3 * MW: 3 * MW + B], in0=t3[:], in1=sq3[:, 0:B],
                op=ALU.subtract,
            )
        with tc.tile_wait_until(2):
            nc.gpsimd.tensor_tensor(
                out=olo[:, B: 2 * B], in0=frsq[:, B: 2 * B],
                in1=frsq[:, 2 * B: 3 * B], op=ALU.add,
            )
            nc.gpsimd.tensor_tensor(
                out=tmp3[:], in0=frsq[:, 3 * B: 4 * B], in1=frsq[:, 0:B],
                op=ALU.subtract,
            )
            nc.gpsimd.tensor_tensor(
                out=olo[:, 0:B], in0=tmp3[:], in1=olo[:, B: 2 * B], op=ALU.subtract
            )
            nc.vector.tensor_tensor(
                out=t2[:], in0=sq2[:, 3 * B: 4 * B], in1=sq2[:, 2 * B: 3 * B],
                op=ALU.subtract,
            )
            nc.vector.tensor_tensor(
                out=t2[:], in0=t2[:], in1=sq2[:, B: 2 * B], op=ALU.subtract
            )
            nc.vector.tensor_tensor(
                out=mt[:, 2 * MW: 2 * MW + B], in0=t2[:], in1=sq2[:, 0:B],
                op=ALU.subtract,
            )

        # --- matmuls: A/B/C1 for all chunks first (DMA-gated only), then the
        # masses-dependent C2s (reusing pair 0's stationary). ---
        def mmABC(c):
            fb = c * FW
            wb = c * PW
            nc.tensor.matmul(
                psA[:], wt[:, wb: wb + 128], ft[:, fb: fb + 512],
                start=c == 0, stop=c == 2,
            )
            nc.tensor.matmul(
                psB[:], wt[:, wb + 128: wb + 256], ft[:, fb + 384: fb + 640],
                start=c == 0, stop=c == 2,
            )
            nc.tensor.matmul(
                psC1[:], wt[:, wb + 256: wb + 320], ft[:, fb + 640: fb + 768],
                start=c == 0, stop=c == 2,
            )

        def mmC2(c, start=False, stop=False):
            nc.tensor.matmul(
                psC2[:, 0:129], wt[:, c * PW: c * PW + 128],
                mt[:, c * MW: c * MW + 129],
                start=start, stop=stop,
            )

        # PE order follows real DMA arrival (ft chunk 3 lands before ft
        # chunk 2, which is last); C2 c0/c1 squeeze in between. Stop flags
        # sit on chunk 2's matmuls (last executed of each group).
        mmABC(0)
        mmABC(1)
        mmABC(3)
        mmC2(0, start=True)
        mmC2(1)
        mmABC(2)
        with tc.tile_wait_until(1):
            mmC2(3)
        with tc.tile_wait_until(2):
            mmC2(2, stop=True)

        # --- quad chain on vector: fr * psA rows is all-additive thanks to
        # the host-side f3 negation; tt adds + two fused stt ops finish ch3
        # (tensor_reduce measured slower than chained tt). ---
        with tc.tile_wait_until(3):
            nc.vector.tensor_tensor(
                out=quad[:], in0=fr[:], in1=psA[0:64, :], op=ALU.mult
            )
            nc.vector.tensor_tensor(
                out=qs[:, 0: 2 * B], in0=quad[:, 0: 2 * B],
                in1=quad[:, 2 * B: 4 * B], op=ALU.add,
            )
            nc.vector.tensor_tensor(
                out=qs[:, 0:B], in0=qs[:, 0:B], in1=qs[:, B: 2 * B], op=ALU.add
            )
        with tc.tile_wait_until(4):
            nc.vector.scalar_tensor_tensor(
                out=tmp3f[:],
                in0=olo[:, 0:B],
                scalar=psC2[0:64, 128:129],
                in1=psC2[0:64, 0:B],
                op0=ALU.mult,
                op1=ALU.add,
            )
            nc.vector.scalar_tensor_tensor(
                out=olo[:, 2 * B: 3 * B],
                in0=qs[:, 0:B],
                scalar=2.0,
                in1=tmp3f[:],
                op0=ALU.mult,
                op1=ALU.add,
            )
        # ch4 = w_pid@f3 and ch6 = w_x1@f5 (low partitions); ch2/ch5 (high)
        nc.scalar.copy(olo[:, 3 * B: 4 * B], psB[0:64, 0:B])
        nc.scalar.copy(olo[:, 4 * B: 5 * B], psC1[:, 0:B])
        nc.scalar.copy(ohi[64:128, 0:B], psA[64:128, 0:B])  # ch2 ener
        nc.scalar.copy(ohi[64:128, B: 2 * B], psB[64:128, B: 2 * B])  # ch5 x0

        # --- DMAs out: one per staging tile, on separate queues ---
        nc.scalar.dma_start(out_d[:, 5 * B: 7 * B], ohi[64:128, :])
        nc.sync.dma_start(out_d[:, 0: 5 * B], olo[:])


_NC_CACHE = {}


def _get_nc():
    if "nc" not in _NC_CACHE:
        nc = bacc.Bacc(
            "TRN2", target_bir_lowering=False, debug=False, num_devices=NCORES
        )
        ft_d = nc.dram_tensor("ft", [128, KC * FW], BF, kind="ExternalInput")
        wt_d = nc.dram_tensor("wt", [128, KC * PW], BF, kind="ExternalInput")
        fr_d = nc.dram_tensor("fr", [64, 4 * B], BF, kind="ExternalInput")
        out_d = nc.dram_tensor("out", [64, 7 * B], DT, kind="ExternalOutput")
        with tile.TileContext(nc) as tc:
            _emit(tc, nc, ft_d.ap(), wt_d.ap(), fr_d.ap(), out_d.ap())
        nc.compile()
        _NC_CACHE["nc"] = nc
    return _NC_CACHE["nc"]


def make_in_maps(combvec, w_dist, w_ener, w_pid, w_extra0, w_extra1):
    ft_t = np.ascontiguousarray(
        np.transpose(np.asarray(combvec, np.float32), (2, 1, 0))
    )  # (6, 512, 128) [k, m, b]
    # ft layout: [p, c*768 + k*128 + b] = ft_t[k, c*128+p, b]
    ftfull = np.ascontiguousarray(
        ft_t.reshape(F, KC, 128, B).transpose(2, 1, 0, 3)
    ).reshape(128, KC * FW)
    ft_np = ftfull.astype(ml_dtypes.bfloat16)

    weights = {
        "w_dist": np.asarray(w_dist, np.float32),
        "w_pid": np.asarray(w_pid, np.float32),
        "w_ener": np.asarray(w_ener, np.float32),
        "w_extra0": np.asarray(w_extra0, np.float32),
        "w_extra1": np.asarray(w_extra1, np.float32),
    }
    in_maps = []
    for core in range(NCORES):
        sl = slice(NS * core, NS * (core + 1))
        # per weight: slice (64, 512) -> transposed chunks (c, p, n) = (4,128,64)
        wch = {
            k: w[sl].T.reshape(KC, 128, NS) for k, w in weights.items()
        }
        # wt layout per chunk: [w_dist|w_ener] (128), [w_pid|w_x0] (128), w_x1 (64)
        wt_ = np.concatenate(
            [
                np.concatenate(
                    [wch["w_dist"], wch["w_ener"], wch["w_pid"],
                     wch["w_extra0"], wch["w_extra1"]], axis=2
                )  # (c, p, 320)
            ],
            axis=2,
        ).transpose(1, 0, 2).reshape(128, KC * PW)
        wt_np = np.ascontiguousarray(wt_).astype(ml_dtypes.bfloat16)
        # fr layout: [p, k*128 + b] = ft_t[k, 64*core+p, b], bf16, f3 negated
        frc = np.ascontiguousarray(ft_t[:4, sl, :].transpose(1, 0, 2)).copy()
        frc[:, 3, :] *= -1.0
        frc_np = frc.reshape(NS, 4 * B).astype(ml_dtypes.bfloat16)
        in_maps.append({"ft": ft_np, "wt": wt_np, "fr": frc_np})
    return in_maps


# out channel order in the DRAM out tensor columns
OUT_ORDER = [0, 1, 3, 4, 6, 2, 5]


def assemble(results):
    full = np.empty((B, N, 7), np.float32)
    for core, r in enumerate(results):
        o = r["out"].reshape(NS, 7, B)  # (n, slot, b)
        for slot, ch in enumerate(OUT_ORDER):
            full[:, NS * core: NS * (core + 1), ch] = o[:, slot, :].T
    return full


def kernel(combvec, w_dist, w_ener, w_pid, w_extra0, w_extra1, _bench=None):
    in_maps = make_in_maps(combvec, w_dist, w_ener, w_pid, w_extra0, w_extra1)
    nc = _get_nc()
    kw = dict(_bench) if _bench else {}
    res = run_bass_kernel_spmd(nc, in_maps, core_ids=list(range(NCORES)), **kw)
    out = assemble(res.results)
    if _bench is not None:
        kernel.last_results = res
    return out


# revision 44
# speedup vs baseline: 1.0077x; 1.0013x over previous
"""LoLa message-passing kernel for 8 Trainium2 NeuronCores.

Math (algebraically identical to the reference):
  ch0 masses      = f3^2 - f0^2 - f1^2 - f2^2
  ch1 ptsq        = f1^2 + f2^2
  ch2 w_ener@f0, ch4 w_pid@f3, ch5 w_extra0@f4, ch6 w_extra1@f5
  ch3 weighted_d  = masses * rowsum(w_dist) + w_dist @ masses
                    + 2*(f0*(w_dist@f0) + f1*(w_dist@f1)
                         + f2*(w_dist@f2) - f3*(w_dist@f3))

Sharding: model-parallel over particles N (64 output rows per core); combvec
replicated (full contraction operand), weights sliced 1/8 per core.

Device-side design notes:
 - Single-term bf16 everywhere (fp32 PSUM accumulate): rel err ~3.6e-3,
   well under the 2e-2 gate, at half the HBM bytes and a third of the
   matmul count of an fp32-faithful hi/lo split.
 - The input stream is byte-rate-bound (~160 GB/s aggregate with all 8
   cores pulling, regardless of queue count or descriptor size). The two
   HWDGE queues are byte-balanced (589/588 KB) and ordered so weights
   land first (every matmul needs them) and ft chunk 2 lands last.
 - Stationary pairs pack two 64-row weight slices side by side; w_dist is
   stored once (C2 reuses pair 0's stationary):
     MM-A: [w_dist | w_ener]  @ [f0|f1|f2|f3]        (512 cols -> psA)
     MM-B: [w_pid  | w_extra0]@ [f3|f4]              (256 cols -> psB)
     MM-C1: w_extra1 (64-col stationary) @ f5        (128 cols -> psC1 lo)
     MM-C2: [w_dist | w_ener] @ [masses|1]           (129 cols -> psC2)
 - masses per 128-row chunk: square + 3 chained subtracts in bf16, written
   straight into the mt tile. Chunks 0+1 as one strided 2-chunk batch on
   vector; chunks 2/3 squares on the scalar ACT engine, combines on
   vector/gpsimd (tensor_reduce measured slower than tt chains).
 - fr (this core's 64 rows of f0..f3) ships bf16 with f3 negated on the
   host, so this core's masses/ptsq squares and the quad combine are
   sign-uniform adds.
 - tc.tile_wait_until stamps force the Tile scheduler's per-engine order
   to match real DMA arrival times (its cost model assumes instant DMAs
   and otherwise schedules the quad chain ahead of the masses combines
   that gate psC2).
"""

import sys

if "/opt/trn_rl_repo" not in sys.path:
    sys.path.insert(0, "/opt/trn_rl_repo")

import numpy as np
import ml_dtypes

import concourse.bass as bass
import concourse.mybir as mybir
import concourse.tile as tile
from concourse import bacc
from concourse.bass_utils import run_bass_kernel_spmd

B, N, F = 128, 512, 6
NCORES = 8
NS = N // NCORES  # 64 output rows per core
KC = N // 128  # 4 contraction chunks of 128
FW = 768  # ft cols per chunk: 6 features x 128 batch
MW = 132  # mt cols per chunk: 128 masses | 1 one | 3 pad
PW = 320  # wt cols per chunk: [w_dist|w_ener] 128, [w_pid|w_x0] 128, w_x1 64
DT = mybir.dt.float32
BF = mybir.dt.bfloat16
ALU = mybir.AluOpType
ACTF = mybir.ActivationFunctionType


def _emit(tc, nc, ft_d, wt_d, fr_d, out_d):
    with (
        tc.tile_pool(name="sbuf", bufs=1) as sb,
        tc.tile_pool(name="scratch", bufs=4) as scr,
        tc.tile_pool(name="psum", bufs=1, space="PSUM") as ps,
    ):
        # --- persistent SBUF tiles ---
        ft = sb.tile([128, KC * FW], BF)  # features [c*768 + k*128 + b]
        mt = sb.tile([128, KC * MW], BF)  # masses [c*132 + b], ones at c*132+128
        wt = sb.tile([128, KC * PW], BF)  # weight pairs [c*320 + ...]
        fr = sb.tile([64, 4 * B], BF)  # this core's n-rows of f0..f3 (f3 negated)
        frsq = sb.tile([64, 4 * B], DT)
        quad = sb.tile([64, 4 * B], BF)  # bf16: 2x DVE rate; feeds 2*qsum only
        qs = sb.tile([64, 2 * B], BF)
        tmp3 = sb.tile([64, B], DT)
        tmp3f = sb.tile([64, B], DT)
        olo = sb.tile([64, 5 * B], DT)  # out staging ch 0,1,3,4,6 (partitions 0:64)
        ohi = sb.tile([128, 2 * B], DT)  # out staging ch 2,5 (partitions 64:128)

        # --- PSUM tiles ---
        psA = ps.tile([128, 512], DT)  # [dist|ener] @ [f0|f1|f2|f3]
        psB = ps.tile([128, 256], DT)  # [pid|x0]   @ [f3|f4]
        psC1 = ps.tile([64, B], DT)  # x1 @ f5 (64-col stationary -> lo partitions)
        psC2 = ps.tile([128, 132], DT)  # [dist|ener] @ [m|1] (dist/rowsum rows :64)

        # --- DMAs in: byte-balanced across the two HWDGE queues (each
        # sustains ~85-110 GB/s; together ~160). sync: ft chunks 0+1 then 3;
        # scalar: fr (feeds the early gpsimd chain), weights, ft chunk 2. ---
        nc.sync.dma_start(ft[:, 0: 2 * FW], ft_d[:, 0: 2 * FW])
        nc.scalar.dma_start(wt[:], wt_d[:])
        nc.scalar.dma_start(fr[:], fr_d[:])
        nc.sync.dma_start(ft[:, 3 * FW:], ft_d[:, 3 * FW:])
        nc.scalar.dma_start(ft[:, 2 * FW: 3 * FW], ft_d[:, 2 * FW: 3 * FW])

        # ones column at [c*MW + 128] (vector's first op; no DMA dep)
        mt4 = mt[:].rearrange("p (c x) -> p c x", c=KC, x=MW)
        nc.vector.memset(mt4[:, :, 128:129], 1.0)

        # --- PE warm-up: dep-free dummy matmuls give HAM sustained activity
        # from kernel start until the weights land (~12us), so the clock
        # ramps (1.2 -> 2.4 GHz) before the real matmul crunch. Also
        # tightens run-to-run clock-state variance. ---
        warm = sb.tile([128, 2 * B], BF)
        psW = ps.tile([128, 512], DT)
        nc.vector.memset(warm[:], 0.5)
        wmov = warm[:, None, :].to_broadcast([128, 4, 2 * B])
        for i in range(7):
            nc.tensor.matmul(
                psW[:], warm[:, 0:B], wmov[:, :, 0:B], start=i == 0, stop=i == 6
            )

        # --- masses: sq = f*f (bf16), m = sq3-sq2-sq1-sq0 chained, last
        # subtract lands in mt. Chunks 0+1 as one strided 2-chunk batch on
        # vector (halves per-op overhead); chunk 2 on vector, chunk 3 on
        # gpsimd (free after the early fr chain). ---
        sq01 = scr.tile([128, 1024], BF, name="sq01")
        t01 = scr.tile([128, 2 * B], BF, name="t01")
        ftv = ft[:, 0: 2 * FW].rearrange("p (cc x) -> p cc x", cc=2, x=FW)
        sqv = sq01[:].rearrange("p (cc x) -> p cc x", cc=2, x=512)
        tv = t01[:].rearrange("p (cc x) -> p cc x", cc=2, x=B)
        mtv = mt[:, 0: 2 * MW].rearrange("p (cc x) -> p cc x", cc=2, x=MW)
        nc.vector.tensor_tensor(
            out=sq01[:], in0=ftv[:, :, 0:512], in1=ftv[:, :, 0:512], op=ALU.mult
        )
        nc.vector.tensor_tensor(
            out=t01[:], in0=sqv[:, :, 3 * B: 4 * B], in1=sqv[:, :, 2 * B: 3 * B],
            op=ALU.subtract,
        )
        nc.vector.tensor_tensor(
            out=t01[:], in0=t01[:], in1=sqv[:, :, B: 2 * B], op=ALU.subtract
        )
        nc.vector.tensor_tensor(
            out=mtv[:, :, 0:B], in0=tv, in1=sqv[:, :, 0:B], op=ALU.subtract
        )
        # chunks 2+3: squares on scalar ACT (otherwise idle mid-kernel);
        # combines on vector (c2) / gpsimd (c3). high_priority hints the
        # scheduler to place the combines ahead of the quad chain, which it
        # otherwise reorders in front of them (its cost model does not see
        # real DMA arrival times).
        sq2 = scr.tile([128, 4 * B], BF, name="sq2")
        t2 = scr.tile([128, B], BF, name="t2")
        sq3 = scr.tile([128, 4 * B], BF, name="sq3")
        t3 = scr.tile([128, B], BF, name="t3")
        # scalar queue order matches real arrival: ft3 (~11.5us) -> fr
        # (~12us) -> ft2 (last, ~14.4us).
        nc.scalar.activation(sq3[:], ft[:, 3 * FW: 3 * FW + 512], ACTF.Square)
        nc.scalar.activation(frsq[:], fr[:], ACTF.Square)
        nc.scalar.activation(sq2[:], ft[:, 2 * FW: 2 * FW + 512], ACTF.Square)
        # c3 combines on gpsimd (ft3 arrives early on the sync queue), then
        # this core's ch0/ch1 combines; the LAST-arriving chunk (ft2) gets
        # its combines on vector ahead of the quad chain.
        with tc.tile_wait_until(1):
            nc.gpsimd.tensor_tensor(
                out=t3[:], in0=sq3[:, 3 * B: 4 * B], in1=sq3[:, 2 * B: 3 * B],
                op=ALU.subtract,
            )
            nc.gpsimd.tensor_tensor(
                out=t3[:], in0=t3[:], in1=sq3[:, B: 2 * B], op=ALU.subtract
            )
            nc.gpsimd.tensor_tensor(
                out=mt[:, 3 * MW: 3 * MW + B], in0=t3[:], in1=sq3[:, 0:B],
                op=ALU.subtract,
            )
        # this core's ch0/ch1 combines fill vector's idle window between
        # m_c01 and the sq2-gated c2 combines (on gpsimd they serialized
        # behind the c3 combines and delayed ch0 past the stt chain).
        with tc.tile_wait_until(2):
            nc.vector.tensor_tensor(
                out=olo[:, B: 2 * B], in0=frsq[:, B: 2 * B],
                in1=frsq[:, 2 * B: 3 * B], op=ALU.add,
            )
            nc.vector.tensor_tensor(
                out=tmp3[:], in0=frsq[:, 3 * B: 4 * B], in1=frsq[:, 0:B],
                op=ALU.subtract,
            )
            nc.vector.tensor_tensor(
                out=olo[:, 0:B], in0=tmp3[:], in1=olo[:, B: 2 * B], op=ALU.subtract
            )
        with tc.tile_wait_until(3):
            nc.vector.tensor_tensor(
                out=t2[:], in0=sq2[:, 3 * B: 4 * B], in1=sq2[:, 2 * B: 3 * B],
                op=ALU.subtract,
            )
            nc.vector.tensor_tensor(
                out=t2[:], in0=t2[:], in1=sq2[:, B: 2 * B], op=ALU.subtract
            )
            nc.vector.tensor_tensor(
                out=mt[:, 2 * MW: 2 * MW + B], in0=t2[:], in1=sq2[:, 0:B],
                op=ALU.subtract,
            )

        # --- matmuls: A/B/C1 for all chunks first (DMA-gated only), then the
        # masses-dependent C2s (reusing pair 0's stationary). ---
        def mmABC(c):
            fb = c * FW
            wb = c * PW
            nc.tensor.matmul(
                psA[:], wt[:, wb: wb + 128], ft[:, fb: fb + 512],
                start=c == 0, stop=c == 2,
            )
            nc.tensor.matmul(
                psB[:], wt[:, wb + 128: wb + 256], ft[:, fb + 384: fb + 640],
                start=c == 0, stop=c == 2,
            )
            nc.tensor.matmul(
                psC1[:], wt[:, wb + 256: wb + 320], ft[:, fb + 640: fb + 768],
                start=c == 0, stop=c == 2,
            )

        def mmC2(c, start=False, stop=False):
            nc.tensor.matmul(
                psC2[:, 0:129], wt[:, c * PW: c * PW + 128],
                mt[:, c * MW: c * MW + 129],
                start=start, stop=stop,
            )

        # PE order follows real DMA arrival (ft chunk 3 lands before ft
        # chunk 2, which is last); C2 c0/c1 squeeze in between. Stop flags
        # sit on chunk 2's matmuls (last executed of each group).
        mmABC(0)
        mmABC(1)
        mmABC(3)
        mmC2(0, start=True)
        mmC2(1)
        mmABC(2)
        with tc.tile_wait_until(1):
            mmC2(3)
        with tc.tile_wait_until(2):
            mmC2(2, stop=True)

        # --- quad chain on vector: fr * psA rows is all-additive thanks to
        # the host-side f3 negation; tt adds + two fused stt ops finish ch3
        # (tensor_reduce measured slower than chained tt). ---
        with tc.tile_wait_until(4):
            nc.vector.tensor_tensor(
                out=quad[:], in0=fr[:], in1=psA[0:64, :], op=ALU.mult
            )
            nc.vector.tensor_tensor(
                out=qs[:, 0: 2 * B], in0=quad[:, 0: 2 * B],
                in1=quad[:, 2 * B: 4 * B], op=ALU.add,
            )
            nc.vector.tensor_tensor(
                out=qs[:, 0:B], in0=qs[:, 0:B], in1=qs[:, B: 2 * B], op=ALU.add
            )
        with tc.tile_wait_until(5):
            nc.vector.scalar_tensor_tensor(
                out=tmp3f[:],
                in0=olo[:, 0:B],
                scalar=psC2[0:64, 128:129],
                in1=psC2[0:64, 0:B],
                op0=ALU.mult,
                op1=ALU.add,
            )
            nc.vector.scalar_tensor_tensor(
                out=olo[:, 2 * B: 3 * B],
                in0=qs[:, 0:B],
                scalar=2.0,
                in1=tmp3f[:],
                op0=ALU.mult,
                op1=ALU.add,
            )
        # ch4 = w_pid@f3 and ch6 = w_x1@f5 (low partitions); ch2/ch5 (high)
        nc.scalar.copy(olo[:, 3 * B: 4 * B], psB[0:64, 0:B])
        nc.scalar.copy(olo[:, 4 * B: 5 * B], psC1[:, 0:B])
        nc.scalar.copy(ohi[64:128, 0:B], psA[64:128, 0:B])  # ch2 ener
        nc.scalar.copy(ohi[64:128, B: 2 * B], psB[64:128, B: 2 * B])  # ch5 x0

        # --- DMAs out: one per staging tile, on separate queues ---
        nc.scalar.dma_start(out_d[:, 5 * B: 7 * B], ohi[64:128, :])
        nc.sync.dma_start(out_d[:, 0: 5 * B], olo[:])


_NC_CACHE = {}


def _get_nc():
    if "nc" not in _NC_CACHE:
        nc = bacc.Bacc(
            "TRN2", target_bir_lowering=False, debug=False, num_devices=NCORES
        )
        ft_d = nc.dram_tensor("ft", [128, KC * FW], BF, kind="ExternalInput")
        wt_d = nc.dram_tensor("wt", [128, KC * PW], BF, kind="ExternalInput")
        fr_d = nc.dram_tensor("fr", [64, 4 * B], BF, kind="ExternalInput")
        out_d = nc.dram_tensor("out", [64, 7 * B], DT, kind="ExternalOutput")
        with tile.TileContext(nc) as tc:
            _emit(tc, nc, ft_d.ap(), wt_d.ap(), fr_d.ap(), out_d.ap())
        nc.compile()
        _NC_CACHE["nc"] = nc
    return _NC_CACHE["nc"]


def make_in_maps(combvec, w_dist, w_ener, w_pid, w_extra0, w_extra1):
    ft_t = np.ascontiguousarray(
        np.transpose(np.asarray(combvec, np.float32), (2, 1, 0))
    )  # (6, 512, 128) [k, m, b]
    # ft layout: [p, c*768 + k*128 + b] = ft_t[k, c*128+p, b]
    ftfull = np.ascontiguousarray(
        ft_t.reshape(F, KC, 128, B).transpose(2, 1, 0, 3)
    ).reshape(128, KC * FW)
    ft_np = ftfull.astype(ml_dtypes.bfloat16)

    weights = {
        "w_dist": np.asarray(w_dist, np.float32),
        "w_pid": np.asarray(w_pid, np.float32),
        "w_ener": np.asarray(w_ener, np.float32),
        "w_extra0": np.asarray(w_extra0, np.float32),
        "w_extra1": np.asarray(w_extra1, np.float32),
    }
    in_maps = []
    for core in range(NCORES):
        sl = slice(NS * core, NS * (core + 1))
        # per weight: slice (64, 512) -> transposed chunks (c, p, n) = (4,128,64)
        wch = {
            k: w[sl].T.reshape(KC, 128, NS) for k, w in weights.items()
        }
        # wt layout per chunk: [w_dist|w_ener] (128), [w_pid|w_x0] (128), w_x1 (64)
        wt_ = np.concatenate(
            [
                np.concatenate(
                    [wch["w_dist"], wch["w_ener"], wch["w_pid"],
                     wch["w_extra0"], wch["w_extra1"]], axis=2
                )  # (c, p, 320)
            ],
            axis=2,
        ).transpose(1, 0, 2).reshape(128, KC * PW)
        wt_np = np.ascontiguousarray(wt_).astype(ml_dtypes.bfloat16)
        # fr layout: [p, k*128 + b] = ft_t[k, 64*core+p, b], bf16, f3 negated
        frc = np.ascontiguousarray(ft_t[:4, sl, :].transpose(1, 0, 2)).copy()
        frc[:, 3, :] *= -1.0
        frc_np = frc.reshape(NS, 4 * B).astype(ml_dtypes.bfloat16)
        in_maps.append({"ft": ft_np, "wt": wt_np, "fr": frc_np})
    return in_maps


# out channel order in the DRAM out tensor columns
OUT_ORDER = [0, 1, 3, 4, 6, 2, 5]


def assemble(results):
    full = np.empty((B, N, 7), np.float32)
    for core, r in enumerate(results):
        o = r["out"].reshape(NS, 7, B)  # (n, slot, b)
        for slot, ch in enumerate(OUT_ORDER):
            full[:, NS * core: NS * (core + 1), ch] = o[:, slot, :].T
    return full


def kernel(combvec, w_dist, w_ener, w_pid, w_extra0, w_extra1, _bench=None):
    in_maps = make_in_maps(combvec, w_dist, w_ener, w_pid, w_extra0, w_extra1)
    nc = _get_nc()
    kw = dict(_bench) if _bench else {}
    res = run_bass_kernel_spmd(nc, in_maps, core_ids=list(range(NCORES)), **kw)
    out = assemble(res.results)
    if _bench is not None:
        kernel.last_results = res
    return out


# revision 45
# speedup vs baseline: 1.0360x; 1.0281x over previous
"""LoLa message-passing kernel for 8 Trainium2 NeuronCores.

Math (algebraically identical to the reference):
  ch0 masses      = f3^2 - f0^2 - f1^2 - f2^2
  ch1 ptsq        = f1^2 + f2^2
  ch2 w_ener@f0, ch4 w_pid@f3, ch5 w_extra0@f4, ch6 w_extra1@f5
  ch3 weighted_d  = masses * rowsum(w_dist) + w_dist @ masses
                    + 2*(f0*(w_dist@f0) + f1*(w_dist@f1)
                         + f2*(w_dist@f2) - f3*(w_dist@f3))

Sharding: model-parallel over particles N (64 output rows per core); combvec
replicated (full contraction operand), weights sliced 1/8 per core.

Device-side design notes:
 - Single-term bf16 everywhere (fp32 PSUM accumulate): rel err ~3.6e-3,
   well under the 2e-2 gate, at half the HBM bytes and a third of the
   matmul count of an fp32-faithful hi/lo split.
 - The input stream is byte-rate-bound (~160 GB/s aggregate with all 8
   cores pulling, regardless of queue count or descriptor size). The two
   HWDGE queues are byte-balanced (589/588 KB) and ordered so weights
   land first (every matmul needs them) and ft chunk 2 lands last.
 - Stationary pairs pack two 64-row weight slices side by side; w_dist is
   stored once (C2 reuses pair 0's stationary):
     MM-A: [w_dist | w_ener]  @ [f0|f1|f2|f3]        (512 cols -> psA)
     MM-B: [w_pid  | w_extra0]@ [f3|f4]              (256 cols -> psB)
     MM-C1: w_extra1 (64-col stationary) @ f5        (128 cols -> psC1 lo)
     MM-C2: [w_dist | w_ener] @ [masses|1]           (129 cols -> psC2)
 - masses per 128-row chunk: square + 3 chained subtracts in bf16, written
   straight into the mt tile. Chunks 0+1 as one strided 2-chunk batch on
   vector; chunks 2/3 squares on the scalar ACT engine, combines on
   vector/gpsimd (tensor_reduce measured slower than tt chains).
 - fr (this core's 64 rows of f0..f3) ships bf16 with f3 negated on the
   host, so this core's masses/ptsq squares and the quad combine are
   sign-uniform adds.
 - tc.tile_wait_until stamps force the Tile scheduler's per-engine order
   to match real DMA arrival times (its cost model assumes instant DMAs
   and otherwise schedules the quad chain ahead of the masses combines
   that gate psC2).
"""

import sys

if "/opt/trn_rl_repo" not in sys.path:
    sys.path.insert(0, "/opt/trn_rl_repo")

import numpy as np
import ml_dtypes

import concourse.bass as bass
import concourse.mybir as mybir
import concourse.tile as tile
from concourse import bacc
from concourse.bass_utils import run_bass_kernel_spmd

B, N, F = 128, 512, 6
NCORES = 8
NS = N // NCORES  # 64 output rows per core
KC = N // 128  # 4 contraction chunks of 128
FW = 768  # ft cols per chunk: 6 features x 128 batch
MW = 132  # mt cols per chunk: 128 masses | 1 one | 3 pad
PW = 320  # wt cols per chunk: [w_dist|w_ener] 128, [w_pid|w_x0] 128, w_x1 64
DT = mybir.dt.float32
BF = mybir.dt.bfloat16
ALU = mybir.AluOpType
ACTF = mybir.ActivationFunctionType


def _emit(tc, nc, ft_d, wt_d, fr_d, out_d):
    with (
        tc.tile_pool(name="sbuf", bufs=1) as sb,
        tc.tile_pool(name="scratch", bufs=4) as scr,
        tc.tile_pool(name="psum", bufs=1, space="PSUM") as ps,
    ):
        # --- persistent SBUF tiles ---
        ft = sb.tile([128, KC * FW], BF)  # features [c*768 + k*128 + b]
        mt = sb.tile([128, KC * MW], BF)  # masses [c*132 + b], ones at c*132+128
        wt = sb.tile([128, KC * PW], BF)  # weight pairs [c*320 + ...]
        fr = sb.tile([64, 4 * B], BF)  # this core's n-rows of f0..f3 (f3 negated)
        frsq = sb.tile([64, 4 * B], DT)
        quad = sb.tile([64, 4 * B], BF)  # bf16: 2x DVE rate; feeds 2*qsum only
        qs = sb.tile([64, 2 * B], BF)
        tmp3 = sb.tile([64, B], DT)
        tmp3f = sb.tile([64, B], DT)
        olo = sb.tile([64, 5 * B], DT)  # out staging ch 0,1,3,4,6 (partitions 0:64)
        ohi = sb.tile([128, 2 * B], DT)  # out staging ch 2,5 (partitions 64:128)

        # --- PSUM tiles ---
        psA = ps.tile([128, 512], DT)  # [dist|ener] @ [f0|f1|f2|f3]
        psB = ps.tile([128, 256], DT)  # [pid|x0]   @ [f3|f4]
        psC1 = ps.tile([64, B], DT)  # x1 @ f5 (64-col stationary -> lo partitions)
        psC2 = ps.tile([128, 132], DT)  # [dist|ener] @ [m|1] (dist/rowsum rows :64)

        # --- DMAs in: byte-balanced across the two HWDGE queues (each
        # sustains ~85-110 GB/s; together ~160). sync: ft chunks 0+1 then 3;
        # scalar: fr (feeds the early gpsimd chain), weights, ft chunk 2. ---
        nc.sync.dma_start(ft[:, 0: 2 * FW], ft_d[:, 0: 2 * FW])
        nc.scalar.dma_start(wt[:], wt_d[:])
        nc.scalar.dma_start(fr[:], fr_d[:])
        nc.sync.dma_start(ft[:, 3 * FW:], ft_d[:, 3 * FW:])
        nc.scalar.dma_start(ft[:, 2 * FW: 3 * FW], ft_d[:, 2 * FW: 3 * FW])

        # ones column at [c*MW + 128] (vector's first op; no DMA dep)
        mt4 = mt[:].rearrange("p (c x) -> p c x", c=KC, x=MW)
        nc.vector.memset(mt4[:, :, 128:129], 1.0)

        # --- PE warm-up: dep-free dummy matmuls give HAM sustained activity
        # from kernel start until the weights land (~12us), so the clock
        # ramps (1.2 -> 2.4 GHz) before the real matmul crunch. Also
        # tightens run-to-run clock-state variance. ---
        warm = sb.tile([128, 2 * B], BF)
        psW = ps.tile([128, 512], DT)
        nc.vector.memset(warm[:], 0.5)
        wmov = warm[:, None, :].to_broadcast([128, 4, 2 * B])
        for i in range(7):
            nc.tensor.matmul(
                psW[:], warm[:, 0:B], wmov[:, :, 0:B], start=i == 0, stop=i == 6
            )

        # --- masses: sq = f*f (bf16), m = sq3-sq2-sq1-sq0 chained, last
        # subtract lands in mt. Chunks 0+1 as one strided 2-chunk batch on
        # vector (halves per-op overhead); chunk 2 on vector, chunk 3 on
        # gpsimd (free after the early fr chain). ---
        sq01 = scr.tile([128, 1024], BF, name="sq01")
        t01 = scr.tile([128, 2 * B], BF, name="t01")
        ftv = ft[:, 0: 2 * FW].rearrange("p (cc x) -> p cc x", cc=2, x=FW)
        sqv = sq01[:].rearrange("p (cc x) -> p cc x", cc=2, x=512)
        tv = t01[:].rearrange("p (cc x) -> p cc x", cc=2, x=B)
        mtv = mt[:, 0: 2 * MW].rearrange("p (cc x) -> p cc x", cc=2, x=MW)
        nc.vector.tensor_tensor(
            out=sq01[:], in0=ftv[:, :, 0:512], in1=ftv[:, :, 0:512], op=ALU.mult
        )
        nc.vector.tensor_tensor(
            out=t01[:], in0=sqv[:, :, 3 * B: 4 * B], in1=sqv[:, :, 2 * B: 3 * B],
            op=ALU.subtract,
        )
        nc.vector.tensor_tensor(
            out=t01[:], in0=t01[:], in1=sqv[:, :, B: 2 * B], op=ALU.subtract
        )
        nc.vector.tensor_tensor(
            out=mtv[:, :, 0:B], in0=tv, in1=sqv[:, :, 0:B], op=ALU.subtract
        )
        # chunks 2+3: squares on scalar ACT (otherwise idle mid-kernel);
        # combines on vector (c2) / gpsimd (c3). high_priority hints the
        # scheduler to place the combines ahead of the quad chain, which it
        # otherwise reorders in front of them (its cost model does not see
        # real DMA arrival times).
        sq2 = scr.tile([128, 4 * B], BF, name="sq2")
        t2 = scr.tile([128, B], BF, name="t2")
        sq3 = scr.tile([128, 4 * B], BF, name="sq3")
        t3 = scr.tile([128, B], BF, name="t3")
        # scalar queue order matches real arrival: ft3 (~11.5us) -> fr
        # (~12us) -> ft2 (last, ~14.4us).
        nc.scalar.activation(sq3[:], ft[:, 3 * FW: 3 * FW + 512], ACTF.Square)
        nc.scalar.activation(frsq[:], fr[:], ACTF.Square)
        nc.scalar.activation(sq2[:], ft[:, 2 * FW: 2 * FW + 512], ACTF.Square)
        # c3 combines on gpsimd (ft3 arrives early on the sync queue), then
        # this core's ch0/ch1 combines; the LAST-arriving chunk (ft2) gets
        # its combines on vector ahead of the quad chain.
        with tc.tile_wait_until(1):
            nc.gpsimd.tensor_tensor(
                out=t3[:], in0=sq3[:, 3 * B: 4 * B], in1=sq3[:, 2 * B: 3 * B],
                op=ALU.subtract,
            )
            nc.gpsimd.tensor_tensor(
                out=t3[:], in0=t3[:], in1=sq3[:, B: 2 * B], op=ALU.subtract
            )
            nc.gpsimd.tensor_tensor(
                out=mt[:, 3 * MW: 3 * MW + B], in0=t3[:], in1=sq3[:, 0:B],
                op=ALU.subtract,
            )
        # this core's ch0/ch1 combines fill vector's idle window between
        # m_c01 and the sq2-gated c2 combines (on gpsimd they serialized
        # behind the c3 combines and delayed ch0 past the stt chain).
        with tc.tile_wait_until(2):
            nc.vector.tensor_tensor(
                out=olo[:, B: 2 * B], in0=frsq[:, B: 2 * B],
                in1=frsq[:, 2 * B: 3 * B], op=ALU.add,
            )
            nc.vector.tensor_tensor(
                out=tmp3[:], in0=frsq[:, 3 * B: 4 * B], in1=frsq[:, 0:B],
                op=ALU.subtract,
            )
            nc.vector.tensor_tensor(
                out=olo[:, 0:B], in0=tmp3[:], in1=olo[:, B: 2 * B], op=ALU.subtract
            )
        with tc.tile_wait_until(3):
            nc.vector.tensor_tensor(
                out=t2[:], in0=sq2[:, 3 * B: 4 * B], in1=sq2[:, 2 * B: 3 * B],
                op=ALU.subtract,
            )
            nc.vector.tensor_tensor(
                out=t2[:], in0=t2[:], in1=sq2[:, B: 2 * B], op=ALU.subtract
            )
            nc.vector.tensor_tensor(
                out=mt[:, 2 * MW: 2 * MW + B], in0=t2[:], in1=sq2[:, 0:B],
                op=ALU.subtract,
            )

        # --- matmuls: A/B/C1 for all chunks first (DMA-gated only), then the
        # masses-dependent C2s (reusing pair 0's stationary). ---
        def mmABC(c):
            fb = c * FW
            wb = c * PW
            nc.tensor.matmul(
                psA[:], wt[:, wb: wb + 128], ft[:, fb: fb + 512],
                start=c == 0, stop=c == 2,
            )
            nc.tensor.matmul(
                psB[:], wt[:, wb + 128: wb + 256], ft[:, fb + 384: fb + 640],
                start=c == 0, stop=c == 2,
            )
            nc.tensor.matmul(
                psC1[:], wt[:, wb + 256: wb + 320], ft[:, fb + 640: fb + 768],
                start=c == 0, stop=c == 2,
            )

        def mmC2(c, start=False, stop=False):
            nc.tensor.matmul(
                psC2[:, 0:129], wt[:, c * PW: c * PW + 128],
                mt[:, c * MW: c * MW + 129],
                start=start, stop=stop,
            )

        # PE order follows real DMA arrival (ft chunk 3 lands before ft
        # chunk 2, which is last); C2 c0/c1 squeeze in between. Stop flags
        # sit on chunk 2's matmuls (last executed of each group).
        mmABC(0)
        mmABC(1)
        mmABC(3)
        mmC2(0, start=True)
        mmC2(1)
        mmABC(2)
        with tc.tile_wait_until(1):
            mmC2(3)
        with tc.tile_wait_until(2):
            mmC2(2, stop=True)

        # --- quad chain on vector: fr * psA rows is all-additive thanks to
        # the host-side f3 negation; tt adds + two fused stt ops finish ch3
        # (tensor_reduce measured slower than chained tt). ---
        with tc.tile_wait_until(4):
            nc.vector.tensor_tensor(
                out=quad[:], in0=fr[:], in1=psA[0:64, :], op=ALU.mult
            )
            nc.vector.tensor_tensor(
                out=qs[:, 0: 2 * B], in0=quad[:, 0: 2 * B],
                in1=quad[:, 2 * B: 4 * B], op=ALU.add,
            )
            nc.vector.tensor_tensor(
                out=qs[:, 0:B], in0=qs[:, 0:B], in1=qs[:, B: 2 * B], op=ALU.add
            )
        with tc.tile_wait_until(5):
            nc.vector.scalar_tensor_tensor(
                out=tmp3f[:],
                in0=olo[:, 0:B],
                scalar=psC2[0:64, 128:129],
                in1=psC2[0:64, 0:B],
                op0=ALU.mult,
                op1=ALU.add,
            )
            nc.vector.scalar_tensor_tensor(
                out=olo[:, 2 * B: 3 * B],
                in0=qs[:, 0:B],
                scalar=2.0,
                in1=tmp3f[:],
                op0=ALU.mult,
                op1=ALU.add,
            )
        # ch4 = w_pid@f3 and ch6 = w_x1@f5 (low partitions); ch2/ch5 (high)
        nc.scalar.copy(olo[:, 3 * B: 4 * B], psB[0:64, 0:B])
        nc.scalar.copy(olo[:, 4 * B: 5 * B], psC1[:, 0:B])
        nc.scalar.copy(ohi[64:128, 0:B], psA[64:128, 0:B])  # ch2 ener
        nc.scalar.copy(ohi[64:128, B: 2 * B], psB[64:128, B: 2 * B])  # ch5 x0

        # --- DMAs out, staggered by readiness on the idle sync queue so the
        # FINAL transfer (whose completion receipt gates the postamble) is
        # just ch3's 32KB: ch0/ch1 go right after the fr combines (~14.5us,
        # input stream already drained), ch4/ch6 after the PSUM copies,
        # ch3 last after stt2. ohi (ch2/ch5) rides the scalar queue. ---
        nc.scalar.dma_start(out_d[:, 5 * B: 7 * B], ohi[64:128, :])
        with tc.tile_wait_until(3):
            nc.sync.dma_start(out_d[:, 0: 2 * B], olo[:, 0: 2 * B])
        with tc.tile_wait_until(5):
            nc.sync.dma_start(out_d[:, 3 * B: 5 * B], olo[:, 3 * B: 5 * B])
        with tc.tile_wait_until(6):
            nc.sync.dma_start(out_d[:, 2 * B: 3 * B], olo[:, 2 * B: 3 * B])


_NC_CACHE = {}


def _get_nc():
    if "nc" not in _NC_CACHE:
        nc = bacc.Bacc(
            "TRN2", target_bir_lowering=False, debug=False, num_devices=NCORES
        )
        ft_d = nc.dram_tensor("ft", [128, KC * FW], BF, kind="ExternalInput")
        wt_d = nc.dram_tensor("wt", [128, KC * PW], BF, kind="ExternalInput")
        fr_d = nc.dram_tensor("fr", [64, 4 * B], BF, kind="ExternalInput")
        out_d = nc.dram_tensor("out", [64, 7 * B], DT, kind="ExternalOutput")
        with tile.TileContext(nc) as tc:
            _emit(tc, nc, ft_d.ap(), wt_d.ap(), fr_d.ap(), out_d.ap())
        nc.compile()
        _NC_CACHE["nc"] = nc
    return _NC_CACHE["nc"]


def make_in_maps(combvec, w_dist, w_ener, w_pid, w_extra0, w_extra1):
    ft_t = np.ascontiguousarray(
        np.transpose(np.asarray(combvec, np.float32), (2, 1, 0))
    )  # (6, 512, 128) [k, m, b]
    # ft layout: [p, c*768 + k*128 + b] = ft_t[k, c*128+p, b]
    ftfull = np.ascontiguousarray(
        ft_t.reshape(F, KC, 128, B).transpose(2, 1, 0, 3)
    ).reshape(128, KC * FW)
    ft_np = ftfull.astype(ml_dtypes.bfloat16)

    weights = {
        "w_dist": np.asarray(w_dist, np.float32),
        "w_pid": np.asarray(w_pid, np.float32),
        "w_ener": np.asarray(w_ener, np.float32),
        "w_extra0": np.asarray(w_extra0, np.float32),
        "w_extra1": np.asarray(w_extra1, np.float32),
    }
    in_maps = []
    for core in range(NCORES):
        sl = slice(NS * core, NS * (core + 1))
        # per weight: slice (64, 512) -> transposed chunks (c, p, n) = (4,128,64)
        wch = {
            k: w[sl].T.reshape(KC, 128, NS) for k, w in weights.items()
        }
        # wt layout per chunk: [w_dist|w_ener] (128), [w_pid|w_x0] (128), w_x1 (64)
        wt_ = np.concatenate(
            [
                np.concatenate(
                    [wch["w_dist"], wch["w_ener"], wch["w_pid"],
                     wch["w_extra0"], wch["w_extra1"]], axis=2
                )  # (c, p, 320)
            ],
            axis=2,
        ).transpose(1, 0, 2).reshape(128, KC * PW)
        wt_np = np.ascontiguousarray(wt_).astype(ml_dtypes.bfloat16)
        # fr layout: [p, k*128 + b] = ft_t[k, 64*core+p, b], bf16, f3 negated
        frc = np.ascontiguousarray(ft_t[:4, sl, :].transpose(1, 0, 2)).copy()
        frc[:, 3, :] *= -1.0
        frc_np = frc.reshape(NS, 4 * B).astype(ml_dtypes.bfloat16)
        in_maps.append({"ft": ft_np, "wt": wt_np, "fr": frc_np})
    return in_maps


# out channel order in the DRAM out tensor columns
OUT_ORDER = [0, 1, 3, 4, 6, 2, 5]


def assemble(results):
    full = np.empty((B, N, 7), np.float32)
    for core, r in enumerate(results):
        o = r["out"].reshape(NS, 7, B)  # (n, slot, b)
        for slot, ch in enumerate(OUT_ORDER):
            full[:, NS * core: NS * (core + 1), ch] = o[:, slot, :].T
    return full


def kernel(combvec, w_dist, w_ener, w_pid, w_extra0, w_extra1, _bench=None):
    in_maps = make_in_maps(combvec, w_dist, w_ener, w_pid, w_extra0, w_extra1)
    nc = _get_nc()
    kw = dict(_bench) if _bench else {}
    res = run_bass_kernel_spmd(nc, in_maps, core_ids=list(range(NCORES)), **kw)
    out = assemble(res.results)
    if _bench is not None:
        kernel.last_results = res
    return out
